# revision 3
# baseline (speedup 1.0000x reference)
"""LSTM decoder (teacher_forcing_ratio=0) on 8 TRN2 NeuronCores.

Strategy
--------
Tensor-parallel over the hidden/gate dimension (each core owns a 256-row
hidden slice = 1024 of the 8192 gate rows), with all state kept transposed
(batch on the SBUF free axis).  The autoregressive feedback
``x_{t+1} = h_t @ W_out.T + b_out`` is folded into the recurrence:

    gates_t = h_{t-1} @ (W_hh + W_ih @ W_out).T + (b + W_ih @ b_out)

so the only cross-core dependency per step is an AllGather of the 256-row
h-slices (fp32r), and the output projection ``out_t = h_t @ W_out.T`` runs
off the critical path, one step behind.  Matmuls run in fp32r (full PE rate
for N>=256, ~tf32 accuracy).  Only ``tgt[:, 0]`` is consumed by the
reference, so just that frame is shipped to the device.
"""

import os

import numpy as np

B, T_FULL, D, H = 512, 128, 128, 2048
NCORES = 8
HS = H // NCORES            # hidden rows per core (256)
MT = (4 * HS) // 128        # gate M-tiles per core (8)
KT = H // 128               # K-slots of the hidden dim (16)

_CACHE = {}


def _build(t_steps):
    import concourse.bacc as bacc
    import concourse.mybir as mybir
    from concourse import tile

    f32 = mybir.dt.float32
    f32r = mybir.dt.float32r
    AF = mybir.ActivationFunctionType

    nc = bacc.Bacc("TRN2", target_bir_lowering=False, debug=False,
                   num_devices=NCORES)

    w_eff = nc.dram_tensor("w_eff", [128, KT * MT * 128], f32r, kind="ExternalInput")
    w_ih = nc.dram_tensor("w_ih", [128, MT * 128], f32r, kind="ExternalInput")
    w_out = nc.dram_tensor("w_out", [128, KT * D], f32r, kind="ExternalInput")
    x0t = nc.dram_tensor("x0t", [128, B], f32r, kind="ExternalInput")
    b0 = nc.dram_tensor("b0", [128, MT], f32, kind="ExternalInput")
    beff = nc.dram_tensor("beff", [128, MT], f32, kind="ExternalInput")
    bout = nc.dram_tensor("bout", [128, 1], f32, kind="ExternalInput")
    out_d = nc.dram_tensor("out", [t_steps, D, B], f32, kind="ExternalOutput")
    inb = nc.dram_tensor("inb", [2 * 128, B], f32r)
    outb = nc.dram_tensor("outb", [KT * 128, B], f32r, addr_space="Shared")

    rg = [list(range(NCORES))]

    with tile.TileContext(nc) as tc:
        with (
            tc.tile_pool(name="w", bufs=1) as wp,
            tc.tile_pool(name="st", bufs=1) as sp,
            tc.tile_pool(name="ot", bufs=2) as op_,
            tc.tile_pool(name="ps", bufs=6, space="PSUM") as ps,
            tc.tile_pool(name="pso", bufs=2, space="PSUM") as pso,
        ):
            w_eff_sb = wp.tile([128, KT * MT * 128], f32r)
            w_ih_sb = wp.tile([128, MT * 128], f32r)
            w_out_sb = wp.tile([128, KT * D], f32r)
            b0_sb = wp.tile([128, MT], f32)
            beff_sb = wp.tile([128, MT], f32)
            bout_sb = wp.tile([128, 1], f32)
            x0_sb = wp.tile([128, B], f32r)

            hT = sp.tile([128, KT * B], f32r)      # gathered h.T (all cores)
            myh = sp.tile([128, 2 * B], f32r)      # this core's h-slice
            cst = sp.tile([128, 2 * B], f32)       # cell state (2 tiles)
            sig = sp.tile([128, MT * B], f32)      # activated gates
            tnc = sp.tile([128, 2 * B], f32)       # tanh(c)
            tmp = sp.tile([128, 2 * B], f32)

            nc.sync.dma_start(w_eff_sb[:], w_eff[:])
            nc.sync.dma_start(w_ih_sb[:], w_ih[:])
            nc.sync.dma_start(w_out_sb[:], w_out[:])
            nc.sync.dma_start(b0_sb[:], b0[:])
            nc.sync.dma_start(beff_sb[:], beff[:])
            nc.sync.dma_start(bout_sb[:], bout[:])
            nc.sync.dma_start(x0_sb[:], x0t[:])

            def emit_wout(t):
                # out_t = h_t @ W_out.T + b_out, from the gathered hT state.
                po = pso.tile([128, B], f32, tag="po")
                for k in range(KT):
                    nc.tensor.matmul(po[:], w_out_sb[:, k * D:(k + 1) * D],
                                     hT[:, k * B:(k + 1) * B],
                                     start=(k == 0), stop=(k == KT - 1))
                ot = op_.tile([128, B], f32, tag="ot")
                nc.scalar.activation(ot[:], po[:], AF.Identity,
                                     bias=bout_sb[:, 0:1])
                nc.sync.dma_start(out_d[t], ot[:])

            for t in range(t_steps):
                # --- gates for step t (reads hT = gathered h_{t-1}) ---
                for m in range(MT):
                    pt = ps.tile([128, B], f32, tag="g")
                    if t == 0:
                        nc.tensor.matmul(pt[:], w_ih_sb[:, m * 128:(m + 1) * 128],
                                         x0_sb[:], start=True, stop=True)
                    else:
                        for k in range(KT):
                            lhsT = w_eff_sb[:, (k * MT + m) * 128:(k * MT + m + 1) * 128]
                            nc.tensor.matmul(pt[:], lhsT, hT[:, k * B:(k + 1) * B],
                                             start=(k == 0), stop=(k == KT - 1))
                    func = AF.Tanh if m in (4, 5) else AF.Sigmoid
                    bias = (b0_sb if t == 0 else beff_sb)[:, m:m + 1]
                    nc.scalar.activation(sig[:, m * B:(m + 1) * B], pt[:], func,
                                         bias=bias)

                # output projection for the previous step overlaps the gather
                if t > 0:
                    emit_wout(t - 1)

                # --- cell/hidden update ---
                for s in range(2):
                    si = sig[:, (0 + s) * B:(1 + s) * B]
                    sf = sig[:, (2 + s) * B:(3 + s) * B]
                    tg = sig[:, (4 + s) * B:(5 + s) * B]
                    so = sig[:, (6 + s) * B:(7 + s) * B]
                    cs = cst[:, s * B:(s + 1) * B]
                    if t == 0:
                        nc.vector.tensor_mul(cs, si, tg)
                    else:
                        nc.vector.tensor_mul(cs, sf, cs)
                        nc.vector.tensor_mul(tmp[:, s * B:(s + 1) * B], si, tg)
                        nc.vector.tensor_add(cs, cs, tmp[:, s * B:(s + 1) * B])
                    nc.scalar.activation(tnc[:, s * B:(s + 1) * B], cs, AF.Tanh)
                    nc.vector.tensor_mul(myh[:, s * B:(s + 1) * B], so,
                                         tnc[:, s * B:(s + 1) * B])

                # --- AllGather the h-slices into hT ---
                nc.sync.dma_start(
                    inb.ap().rearrange("(s p) n -> p s n", s=2),
                    myh[:].rearrange("p (s n) -> p s n", s=2),
                )
                nc.gpsimd.collective_compute(
                    "AllGather", mybir.AluOpType.bypass, replica_groups=rg,
                    ins=[inb.ap().opt()], outs=[outb.ap().opt()],
                )
                nc.sync.dma_start(
                    hT[:].rearrange("p (k n) -> p k n", k=KT),
                    outb.ap().rearrange("(k p) n -> p k n", k=KT),
                )

            emit_wout(t_steps - 1)

    nc.compile()
    return nc


def _prep_inputs(tgt, W_ih, W_hh, b_ih, b_hh, W_out, b_out, t_steps):
    f32 = np.float32
    tgt = np.asarray(tgt, f32)
    W_ih = np.asarray(W_ih, f32)
    W_hh = np.asarray(W_hh, f32)
    W_out = np.asarray(W_out, f32)
    b = np.asarray(b_ih, f32) + np.asarray(b_hh, f32)
    b_out = np.asarray(b_out, f32)

    W_eff = W_hh + W_ih @ W_out          # [4H, H]
    b_eff = b + W_ih @ b_out             # [4H]

    w_out_arr = np.ascontiguousarray(
        W_out.T.reshape(KT, 128, D).transpose(1, 0, 2).reshape(128, KT * D))
    x0t = np.ascontiguousarray(tgt[:, 0, :].T)          # [128, B]
    bout_arr = np.ascontiguousarray(b_out[:, None])     # [128, 1]

    in_maps = []
    for j in range(NCORES):
        rows = np.concatenate(
            [g * H + j * HS + np.arange(HS) for g in range(4)])
        Wj = W_eff[rows]                                 # [1024, H]
        w_eff_arr = np.ascontiguousarray(
            Wj.T.reshape(KT, 128, MT, 128).transpose(1, 0, 2, 3)
            .reshape(128, KT * MT * 128))
        w_ih_arr = np.ascontiguousarray(W_ih[rows].T)    # [128, 1024]
        b0_arr = np.ascontiguousarray(b[rows].reshape(MT, 128).T)
        beff_arr = np.ascontiguousarray(b_eff[rows].reshape(MT, 128).T)
        in_maps.append({
            "w_eff": w_eff_arr, "w_ih": w_ih_arr, "w_out": w_out_arr,
            "x0t": x0t, "b0": b0_arr, "beff": beff_arr, "bout": bout_arr,
        })
    return in_maps


def kernel(tgt, W_ih, W_hh, b_ih, b_hh, W_out, b_out):
    from concourse.bass_utils import run_bass_kernel_spmd

    t_steps = int(os.environ.get("LSTM_T", T_FULL))
    if t_steps not in _CACHE:
        _CACHE[t_steps] = _build(t_steps)
    nc = _CACHE[t_steps]

    in_maps = _prep_inputs(tgt, W_ih, W_hh, b_ih, b_hh, W_out, b_out, t_steps)
    res = run_bass_kernel_spmd(nc, in_maps, core_ids=list(range(NCORES)))
    out = res.results[0]["out"]                    # [t_steps, D, B]
    full = np.ascontiguousarray(out.transpose(2, 0, 1))  # [B, t_steps, D]
    if t_steps == np.asarray(tgt).shape[1]:
        return full
    # debugging path: pad to full length so callers can slice
    return full


# revision 5
# speedup vs baseline: 1.1029x; 1.1029x over previous
"""LSTM decoder (teacher_forcing_ratio=0) on 8 TRN2 NeuronCores.

Strategy
--------
Tensor-parallel over the hidden/gate dimension (each core owns a 256-row
hidden slice = 1024 of the 8192 gate rows), with all state kept transposed
(batch on the SBUF free axis).  The autoregressive feedback
``x_{t+1} = h_t @ W_out.T + b_out`` is folded into the recurrence:

    gates_t = h_{t-1} @ (W_hh + W_ih @ W_out).T + (b + W_ih @ b_out)

so the only cross-core dependency per step is an AllGather of the 256-row
h-slices (fp32r), and the output projection ``out_t = h_t @ W_out.T`` runs
off the critical path, one step behind.  Matmuls run in fp32r (full PE rate
for N>=256, ~tf32 accuracy).  Only ``tgt[:, 0]`` is consumed by the
reference, so just that frame is shipped to the device.
"""

import os

import numpy as np

B, T_FULL, D, H = 512, 128, 128, 2048
NCORES = 8
HS = H // NCORES            # hidden rows per core (256)
MT = (4 * HS) // 128        # gate M-tiles per core (8)
KT = H // 128               # K-slots of the hidden dim (16)

_CACHE = {}


def _build_pipe(t_steps):
    """Half-batch pipelined variant: batch is split into two halves with
    independent recurrences; each half's AllGather overlaps the other
    half's gate matmuls on the PE."""
    import concourse.bacc as bacc
    import concourse.mybir as mybir
    from concourse import tile

    f32 = mybir.dt.float32
    f32r = mybir.dt.float32r
    AF = mybir.ActivationFunctionType
    NB = B // 2  # 256 batch columns per half

    nc = bacc.Bacc("TRN2", target_bir_lowering=False, debug=False,
                   num_devices=NCORES)

    w_eff = nc.dram_tensor("w_eff", [128, KT * MT * 128], f32r, kind="ExternalInput")
    w_ih = nc.dram_tensor("w_ih", [128, MT * 128], f32r, kind="ExternalInput")
    w_out = nc.dram_tensor("w_out", [128, KT * D], f32r, kind="ExternalInput")
    x0t = nc.dram_tensor("x0t", [128, B], f32r, kind="ExternalInput")
    b0 = nc.dram_tensor("b0", [128, MT], f32, kind="ExternalInput")
    beff = nc.dram_tensor("beff", [128, MT], f32, kind="ExternalInput")
    bout = nc.dram_tensor("bout", [128, 1], f32, kind="ExternalInput")
    out_d = nc.dram_tensor("out", [t_steps, D, B], f32, kind="ExternalOutput")
    inb = [nc.dram_tensor(f"inb{h}", [2 * 128, NB], f32r) for h in range(2)]
    outb = [nc.dram_tensor(f"outb{h}", [KT * 128, NB], f32r,
                           addr_space="Shared") for h in range(2)]

    rg = [list(range(NCORES))]

    with tile.TileContext(nc) as tc:
        with (
            tc.tile_pool(name="w", bufs=1) as wp,
            tc.tile_pool(name="st", bufs=1) as sp,
            tc.tile_pool(name="ot", bufs=3) as op_,
            tc.tile_pool(name="ps", bufs=6, space="PSUM") as ps,
            tc.tile_pool(name="pso", bufs=2, space="PSUM") as pso,
        ):
            w_eff_sb = wp.tile([128, KT * MT * 128], f32r)
            w_ih_sb = wp.tile([128, MT * 128], f32r)
            w_out_sb = wp.tile([128, KT * D], f32r)
            b0_sb = wp.tile([128, MT], f32)
            beff_sb = wp.tile([128, MT], f32)
            bout_sb = wp.tile([128, 1], f32)
            x0_sb = wp.tile([128, B], f32r)

            hT = sp.tile([128, KT * B], f32r)  # both halves interleaved per k
            myh = [sp.tile([128, 2 * NB], f32r, tag=f"myh{h}") for h in range(2)]
            cst = [sp.tile([128, 2 * NB], f32, tag=f"c{h}") for h in range(2)]
            sig = [sp.tile([128, MT * NB], f32, tag=f"s{h}") for h in range(2)]
            tnc = [sp.tile([128, 2 * NB], f32, tag=f"tn{h}") for h in range(2)]
            tmp = [sp.tile([128, 2 * NB], f32, tag=f"tm{h}") for h in range(2)]

            nc.sync.dma_start(w_eff_sb[:], w_eff[:])
            nc.sync.dma_start(w_ih_sb[:], w_ih[:])
            nc.sync.dma_start(w_out_sb[:], w_out[:])
            nc.sync.dma_start(b0_sb[:], b0[:])
            nc.sync.dma_start(beff_sb[:], beff[:])
            nc.sync.dma_start(bout_sb[:], bout[:])
            nc.sync.dma_start(x0_sb[:], x0t[:])

            def hT_cols(k, h):
                return hT[:, k * B + h * NB:k * B + h * NB + NB]

            def emit_gates(t, h):
                co = h * NB
                for m in range(MT):
                    pt = ps.tile([128, NB], f32, tag="g")
                    if t == 0:
                        nc.tensor.matmul(pt[:], w_ih_sb[:, m * 128:(m + 1) * 128],
                                         x0_sb[:, co:co + NB],
                                         start=True, stop=True)
                    else:
                        for k in range(KT):
                            lhsT = w_eff_sb[:, (k * MT + m) * 128:(k * MT + m + 1) * 128]
                            nc.tensor.matmul(pt[:], lhsT, hT_cols(k, h),
                                             start=(k == 0), stop=(k == KT - 1))
                    func = AF.Tanh if m in (4, 5) else AF.Sigmoid
                    bias = (b0_sb if t == 0 else beff_sb)[:, m:m + 1]
                    nc.scalar.activation(sig[h][:, m * NB:(m + 1) * NB], pt[:],
                                         func, bias=bias)

            def emit_wout(t, h):
                # out_t(half h) = h_t @ W_out.T + b_out; must be emitted while
                # hT still holds h_t for this half (before the next gather).
                po = pso.tile([128, NB], f32, tag="po")
                for k in range(KT):
                    nc.tensor.matmul(po[:], w_out_sb[:, k * D:(k + 1) * D],
                                     hT_cols(k, h),
                                     start=(k == 0), stop=(k == KT - 1))
                ot = op_.tile([128, NB], f32, tag="ot")
                nc.scalar.activation(ot[:], po[:], AF.Identity,
                                     bias=bout_sb[:, 0:1])
                nc.sync.dma_start(out_d[t][:, h * NB:h * NB + NB], ot[:])

            def emit_update_and_gather(t, h):
                for s in range(2):
                    si = sig[h][:, (0 + s) * NB:(1 + s) * NB]
                    sf = sig[h][:, (2 + s) * NB:(3 + s) * NB]
                    tg = sig[h][:, (4 + s) * NB:(5 + s) * NB]
                    so = sig[h][:, (6 + s) * NB:(7 + s) * NB]
                    cs = cst[h][:, s * NB:(s + 1) * NB]
                    if t == 0:
                        nc.vector.tensor_mul(cs, si, tg)
                    else:
                        nc.vector.tensor_mul(cs, sf, cs)
                        nc.vector.tensor_mul(tmp[h][:, s * NB:(s + 1) * NB], si, tg)
                        nc.vector.tensor_add(cs, cs, tmp[h][:, s * NB:(s + 1) * NB])
                    nc.scalar.activation(tnc[h][:, s * NB:(s + 1) * NB], cs, AF.Tanh)
                    nc.vector.tensor_mul(myh[h][:, s * NB:(s + 1) * NB], so,
                                         tnc[h][:, s * NB:(s + 1) * NB])
                nc.sync.dma_start(
                    inb[h].ap().rearrange("(s p) n -> p s n", s=2),
                    myh[h][:].rearrange("p (s n) -> p s n", s=2),
                )
                nc.gpsimd.collective_compute(
                    "AllGather", mybir.AluOpType.bypass, replica_groups=rg,
                    ins=[inb[h].ap().opt()], outs=[outb[h].ap().opt()],
                )
                for k in range(KT):
                    nc.sync.dma_start(
                        hT_cols(k, h),
                        outb[h].ap()[k * 128:(k + 1) * 128, :],
                    )

            for t in range(t_steps):
                emit_gates(t, 0)
                if t > 0:
                    emit_wout(t - 1, 1)   # h_{t-1} half1 still in hT
                emit_update_and_gather(t, 0)
                emit_gates(t, 1)
                emit_wout(t, 0)           # reads h_t half0 (just gathered)
                emit_update_and_gather(t, 1)

            emit_wout(t_steps - 1, 1)

    nc.compile()
    return nc


def _build(t_steps):
    import concourse.bacc as bacc
    import concourse.mybir as mybir
    from concourse import tile

    f32 = mybir.dt.float32
    f32r = mybir.dt.float32r
    AF = mybir.ActivationFunctionType

    nc = bacc.Bacc("TRN2", target_bir_lowering=False, debug=False,
                   num_devices=NCORES)

    w_eff = nc.dram_tensor("w_eff", [128, KT * MT * 128], f32r, kind="ExternalInput")
    w_ih = nc.dram_tensor("w_ih", [128, MT * 128], f32r, kind="ExternalInput")
    w_out = nc.dram_tensor("w_out", [128, KT * D], f32r, kind="ExternalInput")
    x0t = nc.dram_tensor("x0t", [128, B], f32r, kind="ExternalInput")
    b0 = nc.dram_tensor("b0", [128, MT], f32, kind="ExternalInput")
    beff = nc.dram_tensor("beff", [128, MT], f32, kind="ExternalInput")
    bout = nc.dram_tensor("bout", [128, 1], f32, kind="ExternalInput")
    out_d = nc.dram_tensor("out", [t_steps, D, B], f32, kind="ExternalOutput")
    inb = nc.dram_tensor("inb", [2 * 128, B], f32r)
    outb = nc.dram_tensor("outb", [KT * 128, B], f32r, addr_space="Shared")

    rg = [list(range(NCORES))]

    with tile.TileContext(nc) as tc:
        with (
            tc.tile_pool(name="w", bufs=1) as wp,
            tc.tile_pool(name="st", bufs=1) as sp,
            tc.tile_pool(name="ot", bufs=2) as op_,
            tc.tile_pool(name="ps", bufs=6, space="PSUM") as ps,
            tc.tile_pool(name="pso", bufs=2, space="PSUM") as pso,
        ):
            w_eff_sb = wp.tile([128, KT * MT * 128], f32r)
            w_ih_sb = wp.tile([128, MT * 128], f32r)
            w_out_sb = wp.tile([128, KT * D], f32r)
            b0_sb = wp.tile([128, MT], f32)
            beff_sb = wp.tile([128, MT], f32)
            bout_sb = wp.tile([128, 1], f32)
            x0_sb = wp.tile([128, B], f32r)

            hT = sp.tile([128, KT * B], f32r)      # gathered h.T (all cores)
            myh = sp.tile([128, 2 * B], f32r)      # this core's h-slice
            cst = sp.tile([128, 2 * B], f32)       # cell state (2 tiles)
            sig = sp.tile([128, MT * B], f32)      # activated gates
            tnc = sp.tile([128, 2 * B], f32)       # tanh(c)
            tmp = sp.tile([128, 2 * B], f32)

            nc.sync.dma_start(w_eff_sb[:], w_eff[:])
            nc.sync.dma_start(w_ih_sb[:], w_ih[:])
            nc.sync.dma_start(w_out_sb[:], w_out[:])
            nc.sync.dma_start(b0_sb[:], b0[:])
            nc.sync.dma_start(beff_sb[:], beff[:])
            nc.sync.dma_start(bout_sb[:], bout[:])
            nc.sync.dma_start(x0_sb[:], x0t[:])

            def emit_wout(t):
                # out_t = h_t @ W_out.T + b_out, from the gathered hT state.
                po = pso.tile([128, B], f32, tag="po")
                for k in range(KT):
                    nc.tensor.matmul(po[:], w_out_sb[:, k * D:(k + 1) * D],
                                     hT[:, k * B:(k + 1) * B],
                                     start=(k == 0), stop=(k == KT - 1))
                ot = op_.tile([128, B], f32, tag="ot")
                nc.scalar.activation(ot[:], po[:], AF.Identity,
                                     bias=bout_sb[:, 0:1])
                nc.sync.dma_start(out_d[t], ot[:])

            for t in range(t_steps):
                # --- gates for step t (reads hT = gathered h_{t-1}) ---
                for m in range(MT):
                    pt = ps.tile([128, B], f32, tag="g")
                    if t == 0:
                        nc.tensor.matmul(pt[:], w_ih_sb[:, m * 128:(m + 1) * 128],
                                         x0_sb[:], start=True, stop=True)
                    else:
                        for k in range(KT):
                            lhsT = w_eff_sb[:, (k * MT + m) * 128:(k * MT + m + 1) * 128]
                            nc.tensor.matmul(pt[:], lhsT, hT[:, k * B:(k + 1) * B],
                                             start=(k == 0), stop=(k == KT - 1))
                    func = AF.Tanh if m in (4, 5) else AF.Sigmoid
                    bias = (b0_sb if t == 0 else beff_sb)[:, m:m + 1]
                    nc.scalar.activation(sig[:, m * B:(m + 1) * B], pt[:], func,
                                         bias=bias)

                # output projection for the previous step overlaps the gather
                if t > 0:
                    emit_wout(t - 1)

                # --- cell/hidden update ---
                for s in range(2):
                    si = sig[:, (0 + s) * B:(1 + s) * B]
                    sf = sig[:, (2 + s) * B:(3 + s) * B]
                    tg = sig[:, (4 + s) * B:(5 + s) * B]
                    so = sig[:, (6 + s) * B:(7 + s) * B]
                    cs = cst[:, s * B:(s + 1) * B]
                    if t == 0:
                        nc.vector.tensor_mul(cs, si, tg)
                    else:
                        nc.vector.tensor_mul(cs, sf, cs)
                        nc.vector.tensor_mul(tmp[:, s * B:(s + 1) * B], si, tg)
                        nc.vector.tensor_add(cs, cs, tmp[:, s * B:(s + 1) * B])
                    nc.scalar.activation(tnc[:, s * B:(s + 1) * B], cs, AF.Tanh)
                    nc.vector.tensor_mul(myh[:, s * B:(s + 1) * B], so,
                                         tnc[:, s * B:(s + 1) * B])

                # --- AllGather the h-slices into hT ---
                nc.sync.dma_start(
                    inb.ap().rearrange("(s p) n -> p s n", s=2),
                    myh[:].rearrange("p (s n) -> p s n", s=2),
                )
                nc.gpsimd.collective_compute(
                    "AllGather", mybir.AluOpType.bypass, replica_groups=rg,
                    ins=[inb.ap().opt()], outs=[outb.ap().opt()],
                )
                nc.sync.dma_start(
                    hT[:].rearrange("p (k n) -> p k n", k=KT),
                    outb.ap().rearrange("(k p) n -> p k n", k=KT),
                )

            emit_wout(t_steps - 1)

    nc.compile()
    return nc


def _prep_inputs(tgt, W_ih, W_hh, b_ih, b_hh, W_out, b_out, t_steps):
    f32 = np.float32
    tgt = np.asarray(tgt, f32)
    W_ih = np.asarray(W_ih, f32)
    W_hh = np.asarray(W_hh, f32)
    W_out = np.asarray(W_out, f32)
    b = np.asarray(b_ih, f32) + np.asarray(b_hh, f32)
    b_out = np.asarray(b_out, f32)

    W_eff = W_hh + W_ih @ W_out          # [4H, H]
    b_eff = b + W_ih @ b_out             # [4H]

    w_out_arr = np.ascontiguousarray(
        W_out.T.reshape(KT, 128, D).transpose(1, 0, 2).reshape(128, KT * D))
    x0t = np.ascontiguousarray(tgt[:, 0, :].T)          # [128, B]
    bout_arr = np.ascontiguousarray(b_out[:, None])     # [128, 1]

    in_maps = []
    for j in range(NCORES):
        rows = np.concatenate(
            [g * H + j * HS + np.arange(HS) for g in range(4)])
        Wj = W_eff[rows]                                 # [1024, H]
        w_eff_arr = np.ascontiguousarray(
            Wj.T.reshape(KT, 128, MT, 128).transpose(1, 0, 2, 3)
            .reshape(128, KT * MT * 128))
        w_ih_arr = np.ascontiguousarray(W_ih[rows].T)    # [128, 1024]
        b0_arr = np.ascontiguousarray(b[rows].reshape(MT, 128).T)
        beff_arr = np.ascontiguousarray(b_eff[rows].reshape(MT, 128).T)
        in_maps.append({
            "w_eff": w_eff_arr, "w_ih": w_ih_arr, "w_out": w_out_arr,
            "x0t": x0t, "b0": b0_arr, "beff": beff_arr, "bout": bout_arr,
        })
    return in_maps


def kernel(tgt, W_ih, W_hh, b_ih, b_hh, W_out, b_out):
    from concourse.bass_utils import run_bass_kernel_spmd

    t_steps = int(os.environ.get("LSTM_T", T_FULL))
    pipe = os.environ.get("LSTM_PIPE", "1") == "1"
    key = (t_steps, pipe)
    if key not in _CACHE:
        _CACHE[key] = (_build_pipe if pipe else _build)(t_steps)
    nc = _CACHE[key]

    in_maps = _prep_inputs(tgt, W_ih, W_hh, b_ih, b_hh, W_out, b_out, t_steps)
    res = run_bass_kernel_spmd(nc, in_maps, core_ids=list(range(NCORES)))
    out = res.results[0]["out"]                    # [t_steps, D, B]
    full = np.ascontiguousarray(out.transpose(2, 0, 1))  # [B, t_steps, D]
    if t_steps == np.asarray(tgt).shape[1]:
        return full
    # debugging path: pad to full length so callers can slice
    return full


# revision 6
# speedup vs baseline: 9.5273x; 8.6387x over previous
"""LSTM decoder (teacher_forcing_ratio=0) on 8 TRN2 NeuronCores.

Strategy
--------
Tensor-parallel over the hidden/gate dimension (each core owns a 256-row
hidden slice = 1024 of the 8192 gate rows), with all state kept transposed
(batch on the SBUF free axis).  The autoregressive feedback
``x_{t+1} = h_t @ W_out.T + b_out`` is folded into the recurrence:

    gates_t = h_{t-1} @ (W_hh + W_ih @ W_out).T + (b + W_ih @ b_out)

so the only cross-core dependency per step is an AllGather of the 256-row
h-slices (fp32r), and the output projection ``out_t = h_t @ W_out.T`` runs
off the critical path, one step behind.  Matmuls run in fp32r (full PE rate
for N>=256, ~tf32 accuracy).  Only ``tgt[:, 0]`` is consumed by the
reference, so just that frame is shipped to the device.
"""

import os

import numpy as np

B, T_FULL, D, H = 512, 128, 128, 2048
NCORES = 8
HS = H // NCORES            # hidden rows per core (256)
MT = (4 * HS) // 128        # gate M-tiles per core (8)
KT = H // 128               # K-slots of the hidden dim (16)

_CACHE = {}


def _build_pipe(t_steps):
    """Half-batch pipelined variant: batch is split into two halves with
    independent recurrences; each half's AllGather overlaps the other
    half's gate matmuls on the PE."""
    import concourse.bacc as bacc
    import concourse.mybir as mybir
    from concourse import tile

    f32 = mybir.dt.float32
    f32r = mybir.dt.float32r
    AF = mybir.ActivationFunctionType
    NB = B // 2  # 256 batch columns per half

    nc = bacc.Bacc("TRN2", target_bir_lowering=False, debug=False,
                   num_devices=NCORES)

    w_eff = nc.dram_tensor("w_eff", [128, KT * MT * 128], f32r, kind="ExternalInput")
    w_ih = nc.dram_tensor("w_ih", [128, MT * 128], f32r, kind="ExternalInput")
    w_out = nc.dram_tensor("w_out", [128, KT * D], f32r, kind="ExternalInput")
    x0t = nc.dram_tensor("x0t", [128, B], f32r, kind="ExternalInput")
    b0 = nc.dram_tensor("b0", [128, MT], f32, kind="ExternalInput")
    beff = nc.dram_tensor("beff", [128, MT], f32, kind="ExternalInput")
    bout = nc.dram_tensor("bout", [128, 1], f32, kind="ExternalInput")
    out_d = nc.dram_tensor("out", [t_steps, D, B], f32, kind="ExternalOutput")
    inb = [nc.dram_tensor(f"inb{h}", [2 * 128, NB], f32r) for h in range(2)]
    outb = [nc.dram_tensor(f"outb{h}", [KT * 128, NB], f32r,
                           addr_space="Shared") for h in range(2)]

    rg = [list(range(NCORES))]

    with tile.TileContext(nc) as tc:
        with (
            tc.tile_pool(name="w", bufs=1) as wp,
            tc.tile_pool(name="st", bufs=1) as sp,
            tc.tile_pool(name="ot", bufs=3) as op_,
            tc.tile_pool(name="ps", bufs=6, space="PSUM") as ps,
            tc.tile_pool(name="pso", bufs=2, space="PSUM") as pso,
        ):
            w_eff_sb = wp.tile([128, KT * MT * 128], f32r)
            w_ih_sb = wp.tile([128, MT * 128], f32r)
            w_out_sb = wp.tile([128, KT * D], f32r)
            b0_sb = wp.tile([128, MT], f32)
            beff_sb = wp.tile([128, MT], f32)
            bout_sb = wp.tile([128, 1], f32)
            x0_sb = wp.tile([128, B], f32r)

            hT = sp.tile([128, KT * B], f32r)  # both halves interleaved per k
            myh = [sp.tile([128, 2 * NB], f32r, name=f"myh{h}", tag=f"myh{h}") for h in range(2)]
            cst = [sp.tile([128, 2 * NB], f32, name=f"c{h}", tag=f"c{h}") for h in range(2)]
            sig = [sp.tile([128, MT * NB], f32, name=f"s{h}", tag=f"s{h}") for h in range(2)]
            tnc = [sp.tile([128, 2 * NB], f32, name=f"tn{h}", tag=f"tn{h}") for h in range(2)]
            tmp = [sp.tile([128, 2 * NB], f32, name=f"tm{h}", tag=f"tm{h}") for h in range(2)]

            nc.sync.dma_start(w_eff_sb[:], w_eff[:])
            nc.sync.dma_start(w_ih_sb[:], w_ih[:])
            nc.sync.dma_start(w_out_sb[:], w_out[:])
            nc.sync.dma_start(b0_sb[:], b0[:])
            nc.sync.dma_start(beff_sb[:], beff[:])
            nc.sync.dma_start(bout_sb[:], bout[:])
            nc.sync.dma_start(x0_sb[:], x0t[:])

            def hT_cols(k, h):
                return hT[:, k * B + h * NB:k * B + h * NB + NB]

            def emit_gates(t, h):
                co = h * NB
                for m in range(MT):
                    pt = ps.tile([128, NB], f32, tag="g")
                    if t == 0:
                        nc.tensor.matmul(pt[:], w_ih_sb[:, m * 128:(m + 1) * 128],
                                         x0_sb[:, co:co + NB],
                                         start=True, stop=True)
                    else:
                        for k in range(KT):
                            lhsT = w_eff_sb[:, (k * MT + m) * 128:(k * MT + m + 1) * 128]
                            nc.tensor.matmul(pt[:], lhsT, hT_cols(k, h),
                                             start=(k == 0), stop=(k == KT - 1))
                    func = AF.Tanh if m in (4, 5) else AF.Sigmoid
                    bias = (b0_sb if t == 0 else beff_sb)[:, m:m + 1]
                    nc.scalar.activation(sig[h][:, m * NB:(m + 1) * NB], pt[:],
                                         func, bias=bias)

            def emit_wout(t, h):
                # out_t(half h) = h_t @ W_out.T + b_out; must be emitted while
                # hT still holds h_t for this half (before the next gather).
                po = pso.tile([128, NB], f32, tag="po")
                for k in range(KT):
                    nc.tensor.matmul(po[:], w_out_sb[:, k * D:(k + 1) * D],
                                     hT_cols(k, h),
                                     start=(k == 0), stop=(k == KT - 1))
                ot = op_.tile([128, NB], f32, tag="ot")
                nc.scalar.activation(ot[:], po[:], AF.Identity,
                                     bias=bout_sb[:, 0:1])
                nc.sync.dma_start(out_d[t][:, h * NB:h * NB + NB], ot[:])

            def emit_update_and_gather(t, h):
                for s in range(2):
                    si = sig[h][:, (0 + s) * NB:(1 + s) * NB]
                    sf = sig[h][:, (2 + s) * NB:(3 + s) * NB]
                    tg = sig[h][:, (4 + s) * NB:(5 + s) * NB]
                    so = sig[h][:, (6 + s) * NB:(7 + s) * NB]
                    cs = cst[h][:, s * NB:(s + 1) * NB]
                    if t == 0:
                        nc.vector.tensor_mul(cs, si, tg)
                    else:
                        nc.vector.tensor_mul(cs, sf, cs)
                        nc.vector.tensor_mul(tmp[h][:, s * NB:(s + 1) * NB], si, tg)
                        nc.vector.tensor_add(cs, cs, tmp[h][:, s * NB:(s + 1) * NB])
                    nc.scalar.activation(tnc[h][:, s * NB:(s + 1) * NB], cs, AF.Tanh)
                    nc.vector.tensor_mul(myh[h][:, s * NB:(s + 1) * NB], so,
                                         tnc[h][:, s * NB:(s + 1) * NB])
                nc.sync.dma_start(
                    inb[h].ap().rearrange("(s p) n -> p s n", s=2),
                    myh[h][:].rearrange("p (s n) -> p s n", s=2),
                )
                nc.gpsimd.collective_compute(
                    "AllGather", mybir.AluOpType.bypass, replica_groups=rg,
                    ins=[inb[h].ap().opt()], outs=[outb[h].ap().opt()],
                )
                for k in range(KT):
                    nc.sync.dma_start(
                        hT_cols(k, h),
                        outb[h].ap()[k * 128:(k + 1) * 128, :],
                    )

            for t in range(t_steps):
                emit_gates(t, 0)
                if t > 0:
                    emit_wout(t - 1, 1)   # h_{t-1} half1 still in hT
                emit_update_and_gather(t, 0)
                emit_gates(t, 1)
                emit_wout(t, 0)           # reads h_t half0 (just gathered)
                emit_update_and_gather(t, 1)

            emit_wout(t_steps - 1, 1)

    nc.compile()
    return nc


def _build(t_steps):
    import concourse.bacc as bacc
    import concourse.mybir as mybir
    from concourse import tile

    f32 = mybir.dt.float32
    f32r = mybir.dt.float32r
    AF = mybir.ActivationFunctionType

    nc = bacc.Bacc("TRN2", target_bir_lowering=False, debug=False,
                   num_devices=NCORES)

    w_eff = nc.dram_tensor("w_eff", [128, KT * MT * 128], f32r, kind="ExternalInput")
    w_ih = nc.dram_tensor("w_ih", [128, MT * 128], f32r, kind="ExternalInput")
    w_out = nc.dram_tensor("w_out", [128, KT * D], f32r, kind="ExternalInput")
    x0t = nc.dram_tensor("x0t", [128, B], f32r, kind="ExternalInput")
    b0 = nc.dram_tensor("b0", [128, MT], f32, kind="ExternalInput")
    beff = nc.dram_tensor("beff", [128, MT], f32, kind="ExternalInput")
    bout = nc.dram_tensor("bout", [128, 1], f32, kind="ExternalInput")
    out_d = nc.dram_tensor("out", [t_steps, D, B], f32, kind="ExternalOutput")
    inb = nc.dram_tensor("inb", [2 * 128, B], f32r)
    outb = nc.dram_tensor("outb", [KT * 128, B], f32r, addr_space="Shared")

    rg = [list(range(NCORES))]

    with tile.TileContext(nc) as tc:
        with (
            tc.tile_pool(name="w", bufs=1) as wp,
            tc.tile_pool(name="st", bufs=1) as sp,
            tc.tile_pool(name="ot", bufs=2) as op_,
            tc.tile_pool(name="ps", bufs=6, space="PSUM") as ps,
            tc.tile_pool(name="pso", bufs=2, space="PSUM") as pso,
        ):
            w_eff_sb = wp.tile([128, KT * MT * 128], f32r)
            w_ih_sb = wp.tile([128, MT * 128], f32r)
            w_out_sb = wp.tile([128, KT * D], f32r)
            b0_sb = wp.tile([128, MT], f32)
            beff_sb = wp.tile([128, MT], f32)
            bout_sb = wp.tile([128, 1], f32)
            x0_sb = wp.tile([128, B], f32r)

            hT = sp.tile([128, KT * B], f32r)      # gathered h.T (all cores)
            myh = sp.tile([128, 2 * B], f32r)      # this core's h-slice
            cst = sp.tile([128, 2 * B], f32)       # cell state (2 tiles)
            sig = sp.tile([128, MT * B], f32)      # activated gates
            tnc = sp.tile([128, 2 * B], f32)       # tanh(c)
            tmp = sp.tile([128, 2 * B], f32)

            nc.sync.dma_start(w_eff_sb[:], w_eff[:])
            nc.sync.dma_start(w_ih_sb[:], w_ih[:])
            nc.sync.dma_start(w_out_sb[:], w_out[:])
            nc.sync.dma_start(b0_sb[:], b0[:])
            nc.sync.dma_start(beff_sb[:], beff[:])
            nc.sync.dma_start(bout_sb[:], bout[:])
            nc.sync.dma_start(x0_sb[:], x0t[:])

            def emit_wout(t):
                # out_t = h_t @ W_out.T + b_out, from the gathered hT state.
                po = pso.tile([128, B], f32, tag="po")
                for k in range(KT):
                    nc.tensor.matmul(po[:], w_out_sb[:, k * D:(k + 1) * D],
                                     hT[:, k * B:(k + 1) * B],
                                     start=(k == 0), stop=(k == KT - 1))
                ot = op_.tile([128, B], f32, tag="ot")
                nc.scalar.activation(ot[:], po[:], AF.Identity,
                                     bias=bout_sb[:, 0:1])
                nc.sync.dma_start(out_d[t], ot[:])

            for t in range(t_steps):
                # --- gates for step t (reads hT = gathered h_{t-1}) ---
                for m in range(MT):
                    pt = ps.tile([128, B], f32, tag="g")
                    if t == 0:
                        nc.tensor.matmul(pt[:], w_ih_sb[:, m * 128:(m + 1) * 128],
                                         x0_sb[:], start=True, stop=True)
                    else:
                        for k in range(KT):
                            lhsT = w_eff_sb[:, (k * MT + m) * 128:(k * MT + m + 1) * 128]
                            nc.tensor.matmul(pt[:], lhsT, hT[:, k * B:(k + 1) * B],
                                             start=(k == 0), stop=(k == KT - 1))
                    func = AF.Tanh if m in (4, 5) else AF.Sigmoid
                    bias = (b0_sb if t == 0 else beff_sb)[:, m:m + 1]
                    nc.scalar.activation(sig[:, m * B:(m + 1) * B], pt[:], func,
                                         bias=bias)

                # output projection for the previous step overlaps the gather
                if t > 0:
                    emit_wout(t - 1)

                # --- cell/hidden update ---
                for s in range(2):
                    si = sig[:, (0 + s) * B:(1 + s) * B]
                    sf = sig[:, (2 + s) * B:(3 + s) * B]
                    tg = sig[:, (4 + s) * B:(5 + s) * B]
                    so = sig[:, (6 + s) * B:(7 + s) * B]
                    cs = cst[:, s * B:(s + 1) * B]
                    if t == 0:
                        nc.vector.tensor_mul(cs, si, tg)
                    else:
                        nc.vector.tensor_mul(cs, sf, cs)
                        nc.vector.tensor_mul(tmp[:, s * B:(s + 1) * B], si, tg)
                        nc.vector.tensor_add(cs, cs, tmp[:, s * B:(s + 1) * B])
                    nc.scalar.activation(tnc[:, s * B:(s + 1) * B], cs, AF.Tanh)
                    nc.vector.tensor_mul(myh[:, s * B:(s + 1) * B], so,
                                         tnc[:, s * B:(s + 1) * B])

                # --- AllGather the h-slices into hT ---
                nc.sync.dma_start(
                    inb.ap().rearrange("(s p) n -> p s n", s=2),
                    myh[:].rearrange("p (s n) -> p s n", s=2),
                )
                nc.gpsimd.collective_compute(
                    "AllGather", mybir.AluOpType.bypass, replica_groups=rg,
                    ins=[inb.ap().opt()], outs=[outb.ap().opt()],
                )
                nc.sync.dma_start(
                    hT[:].rearrange("p (k n) -> p k n", k=KT),
                    outb.ap().rearrange("(k p) n -> p k n", k=KT),
                )

            emit_wout(t_steps - 1)

    nc.compile()
    return nc


def _prep_inputs(tgt, W_ih, W_hh, b_ih, b_hh, W_out, b_out, t_steps):
    f32 = np.float32
    tgt = np.asarray(tgt, f32)
    W_ih = np.asarray(W_ih, f32)
    W_hh = np.asarray(W_hh, f32)
    W_out = np.asarray(W_out, f32)
    b = np.asarray(b_ih, f32) + np.asarray(b_hh, f32)
    b_out = np.asarray(b_out, f32)

    W_eff = W_hh + W_ih @ W_out          # [4H, H]
    b_eff = b + W_ih @ b_out             # [4H]

    w_out_arr = np.ascontiguousarray(
        W_out.T.reshape(KT, 128, D).transpose(1, 0, 2).reshape(128, KT * D))
    x0t = np.ascontiguousarray(tgt[:, 0, :].T)          # [128, B]
    bout_arr = np.ascontiguousarray(b_out[:, None])     # [128, 1]

    in_maps = []
    for j in range(NCORES):
        rows = np.concatenate(
            [g * H + j * HS + np.arange(HS) for g in range(4)])
        Wj = W_eff[rows]                                 # [1024, H]
        w_eff_arr = np.ascontiguousarray(
            Wj.T.reshape(KT, 128, MT, 128).transpose(1, 0, 2, 3)
            .reshape(128, KT * MT * 128))
        w_ih_arr = np.ascontiguousarray(W_ih[rows].T)    # [128, 1024]
        b0_arr = np.ascontiguousarray(b[rows].reshape(MT, 128).T)
        beff_arr = np.ascontiguousarray(b_eff[rows].reshape(MT, 128).T)
        in_maps.append({
            "w_eff": w_eff_arr, "w_ih": w_ih_arr, "w_out": w_out_arr,
            "x0t": x0t, "b0": b0_arr, "beff": beff_arr, "bout": bout_arr,
        })
    return in_maps


def kernel(tgt, W_ih, W_hh, b_ih, b_hh, W_out, b_out):
    from concourse.bass_utils import run_bass_kernel_spmd

    t_steps = int(os.environ.get("LSTM_T", T_FULL))
    pipe = os.environ.get("LSTM_PIPE", "1") == "1"
    key = (t_steps, pipe)
    if key not in _CACHE:
        _CACHE[key] = (_build_pipe if pipe else _build)(t_steps)
    nc = _CACHE[key]

    in_maps = _prep_inputs(tgt, W_ih, W_hh, b_ih, b_hh, W_out, b_out, t_steps)
    res = run_bass_kernel_spmd(nc, in_maps, core_ids=list(range(NCORES)))
    out = res.results[0]["out"]                    # [t_steps, D, B]
    full = np.ascontiguousarray(out.transpose(2, 0, 1))  # [B, t_steps, D]
    if t_steps == np.asarray(tgt).shape[1]:
        return full
    # debugging path: pad to full length so callers can slice
    return full
